# revision 1
# baseline (speedup 1.0000x reference)
"""Complex GRU cell on 8 Trainium2 NeuronCores (Bass/Tile).

Strategy
  - Data-parallel: batch 16384 -> 8 cores x 2048; 512x512 weights replicated.
  - Feature-major (transposed) layout on device: host pre-transposes x,h to
    [D, B_local] fp16 and pre-transposes the weights, so the kernel needs no
    on-device transposes and per-feature biases land on SBUF partitions
    (fused into ACT ops).
  - Complex matmul uses the Gauss 3-multiplication trick: with host-prepared
    weight variants Wr, (Wi-Wr), -(Wr+Wi) and input sums (Xr+Xi),
      C = (Xr+Xi)@Wr, A = Xi@(-(Wr+Wi)), B = Xr@(Wi-Wr)
      re = C + A, im = C + B
    i.e. 3 fp16 matmul groups (fp32 PSUM accumulate) + one ACT copy + two DVE
    adds, instead of 4 matmul groups -- 25% less TensorE work.
  - Per batch-chunk of 512: R wave (sigmoid gate), r*h on DVE, Z wave,
    C wave (x part + (r*h) part), polar-tanh via ACT Sqrt/Tanh plus
    1/|c| = exp(-0.5 ln |c|^2) on ACT, final complex blend h + z*(h~ - h).
"""
import sys

for _p in ("/opt/trn_rl_repo",):
    if _p not in sys.path:
        sys.path.insert(0, _p)

import numpy as np
import concourse.bass as bass
import concourse.tile as tile
import concourse.mybir as mybir
from concourse.bass_utils import run_bass_kernel_spmd

F32, F16 = mybir.dt.float32, mybir.dt.float16
AF = mybir.ActivationFunctionType
ALU = mybir.AluOpType

RE, IM, IMN = 0, 1, 2  # weight variant slots: Wr, (Wi-Wr), -(Wr+Wi)
GZ, GR, GH = 0, 1, 2   # gates (z, r, candidate)

N_CORES = 8
B_FULL, D, H = 16384, 512, 512
B_LOCAL = B_FULL // N_CORES
BCHUNK = 512

LAST_RUN_INFO = {}
_CACHE = {}


def _split_waits(nc, maxw=1):
    """walrus here allows 1 sync wait per instruction; hoist extras onto NoOps."""
    for fn in nc.m.functions:
        for bb in fn.blocks:
            out = []
            for inst in list(bb.instructions):
                si = inst.sync_info
                waits = list(si.on_wait) if si is not None else []
                if len(waits) > maxw:
                    extra, keep = waits[:-maxw], waits[-maxw:]
                    k = 0
                    while extra:
                        chunk, extra = extra[:maxw], extra[maxw:]
                        out.append(mybir.InstNoOp(
                            name=f"{inst.name}-wsplit{k}", engine=inst.engine,
                            ins=[], outs=[],
                            sync_info=mybir.SyncInfo(on_wait=chunk, on_update=[])))
                        k += 1
                    inst.sync_info = mybir.SyncInfo(on_wait=keep,
                                                    on_update=list(si.on_update))
                out.append(inst)
            bb.instructions[:] = out
    return nc


def _build(split_for_hw=True):
    NBC = B_LOCAL // BCHUNK
    nc = bass.Bass("TRN2", target_bir_lowering=False, debug=False)

    dram_acts = {}
    for nm in ("xr", "xi", "xs", "hr", "hi", "hs"):
        dram_acts[nm] = nc.dram_tensor(nm, [4, 128, B_LOCAL], F16,
                                       kind="ExternalInput")
    wx = nc.dram_tensor("wx", [3, 3, 4, 128, 512], F16, kind="ExternalInput")
    wh = nc.dram_tensor("wh", [3, 3, 4, 128, 512], F16, kind="ExternalInput")
    bias = nc.dram_tensor("bias", [3, 2, 4, 128], F32, kind="ExternalInput")
    outr = nc.dram_tensor("outr", [512, B_LOCAL], F32, kind="ExternalOutput")
    outi = nc.dram_tensor("outi", [512, B_LOCAL], F32, kind="ExternalOutput")

    with tile.TileContext(nc) as tc:
        with (
            tc.tile_pool(name="wpool", bufs=1) as wpool,
            tc.tile_pool(name="apool", bufs=2) as apool,
            tc.tile_pool(name="rhpool", bufs=1) as rhpool,
            tc.tile_pool(name="zpool", bufs=1) as zpool,
            tc.tile_pool(name="spool", bufs=2) as spool,
            tc.tile_pool(name="opool", bufs=1) as opool,
            tc.tile_pool(name="cpool", bufs=1) as cpool,
            tc.tile_pool(name="pspool", bufs=8, space="PSUM") as pspool,
        ):
            W = {}
            for which, src in (("x", wx), ("h", wh)):
                for g in range(3):
                    for v in range(3):
                        for dt in range(4):
                            t = wpool.tile([128, 512], F16, tag=f"w{which}{g}{v}{dt}")
                            nc.sync.dma_start(t[:], src[g, v, dt])
                            W[(which, g, v, dt)] = t
            BT = {}
            for g in range(3):
                for comp in range(2):
                    for t4 in range(4):
                        t = wpool.tile([128, 1], F32, tag=f"b{g}{comp}{t4}")
                        nc.sync.dma_start(
                            t[:], bias[g, comp, t4].rearrange("(p o) -> p o", o=1))
                        BT[(g, comp, t4)] = t

            def bank_mms(ps, g, v, srcs, t4, first_start, last_stop=True):
                """One Gauss product group accumulated into psum tile ps.
                srcs: list of ("x"|"h", act_tile_dict)."""
                n = len(srcs) * 4
                i = 0
                for which, act in srcs:
                    for dt in range(4):
                        nc.tensor.matmul(
                            ps[:],
                            W[(which, g, v, dt)][:, t4 * 128:(t4 + 1) * 128],
                            act[dt][:],
                            start=(first_start and i == 0),
                            stop=(last_stop and i == n - 1))
                        i += 1

            def gauss_combine(A, Bk, C, tagp):
                """re = C + A, im = C + B via one ACT copy + two DVE adds."""
                csb = spool.tile([128, BCHUNK], F16, tag="csb")
                nc.scalar.activation(csb[:], C[:], AF.Identity)
                pre_r = spool.tile([128, BCHUNK], F16, tag=f"{tagp}r")
                pre_i = spool.tile([128, BCHUNK], F16, tag=f"{tagp}i")
                nc.vector.tensor_tensor(pre_r[:], A[:], csb[:], ALU.add)
                nc.vector.tensor_tensor(pre_i[:], Bk[:], csb[:], ALU.add)
                return pre_r, pre_i

            for bc in range(NBC):
                bsl = slice(bc * BCHUNK, (bc + 1) * BCHUNK)
                act = {}
                for nm in ("xr", "xi", "xs", "hr", "hi", "hs"):
                    d = {}
                    for dt in range(4):
                        t = apool.tile([128, BCHUNK], F16, tag=f"a{nm}{dt}")
                        nc.sync.dma_start(t[:], dram_acts[nm][dt, :, bsl])
                        d[dt] = t
                    act[nm] = d

                # R wave: r = cv_sigmoid(px1 + pr); rh = r*h (+ sum for Gauss)
                rh_re, rh_im, rh_s = {}, {}, {}
                for t4 in range(4):
                    A = pspool.tile([128, BCHUNK], F32, tag="ps")
                    bank_mms(A, GR, IMN, [("x", act["xi"]), ("h", act["hi"])], t4, True)
                    Bk = pspool.tile([128, BCHUNK], F32, tag="ps")
                    bank_mms(Bk, GR, IM, [("x", act["xr"]), ("h", act["hr"])], t4, True)
                    C = pspool.tile([128, BCHUNK], F32, tag="ps")
                    bank_mms(C, GR, RE, [("x", act["xs"]), ("h", act["hs"])], t4, True)
                    pre_r, pre_i = gauss_combine(A, Bk, C, "rp")
                    rr = spool.tile([128, BCHUNK], F16, tag="r0")
                    ri = spool.tile([128, BCHUNK], F16, tag="r1")
                    nc.scalar.activation(rr[:], pre_r[:], AF.Sigmoid,
                                         bias=BT[(GR, 0, t4)][:])
                    nc.scalar.activation(ri[:], pre_i[:], AF.Sigmoid,
                                         bias=BT[(GR, 1, t4)][:])
                    t1 = spool.tile([128, BCHUNK], F16, tag="t1")
                    t2 = spool.tile([128, BCHUNK], F16, tag="t2")
                    nc.vector.tensor_tensor(t1[:], rr[:], act["hr"][t4][:], ALU.mult)
                    nc.vector.tensor_tensor(t2[:], ri[:], act["hi"][t4][:], ALU.mult)
                    rhr = rhpool.tile([128, BCHUNK], F16, tag=f"rhr{t4}")
                    nc.vector.tensor_tensor(rhr[:], t1[:], t2[:], ALU.subtract)
                    t3 = spool.tile([128, BCHUNK], F16, tag="t1")
                    t4b = spool.tile([128, BCHUNK], F16, tag="t2")
                    nc.vector.tensor_tensor(t3[:], rr[:], act["hi"][t4][:], ALU.mult)
                    nc.vector.tensor_tensor(t4b[:], ri[:], act["hr"][t4][:], ALU.mult)
                    rhi = rhpool.tile([128, BCHUNK], F16, tag=f"rhi{t4}")
                    nc.vector.tensor_tensor(rhi[:], t3[:], t4b[:], ALU.add)
                    rhs = rhpool.tile([128, BCHUNK], F16, tag=f"rhs{t4}")
                    nc.vector.tensor_tensor(rhs[:], rhr[:], rhi[:], ALU.add)
                    rh_re[t4], rh_im[t4], rh_s[t4] = rhr, rhi, rhs

                # Z wave: z = cv_sigmoid(px0 + pz)
                z16 = {}
                for t4 in range(4):
                    A = pspool.tile([128, BCHUNK], F32, tag="ps")
                    bank_mms(A, GZ, IMN, [("x", act["xi"]), ("h", act["hi"])], t4, True)
                    Bk = pspool.tile([128, BCHUNK], F32, tag="ps")
                    bank_mms(Bk, GZ, IM, [("x", act["xr"]), ("h", act["hr"])], t4, True)
                    C = pspool.tile([128, BCHUNK], F32, tag="ps")
                    bank_mms(C, GZ, RE, [("x", act["xs"]), ("h", act["hs"])], t4, True)
                    pre_r, pre_i = gauss_combine(A, Bk, C, "zp")
                    for comp, pre in ((0, pre_r), (1, pre_i)):
                        zt = zpool.tile([128, BCHUNK], F16, tag=f"z{t4}{comp}")
                        nc.scalar.activation(zt[:], pre[:], AF.Sigmoid,
                                             bias=BT[(GZ, comp, t4)][:])
                        z16[(t4, comp)] = zt

                # C wave: c = px2 + (r*h)@Wh2^T; h_tilde = polar_tanh(c + b)
                # alpha pass: everything through Ln/Exp (one table-set switch
                # into natural_log_exp; Identity/Square are fillers in every
                # set so they don't thrash the ACT tables)
                cb16, mag16, inv16 = {}, {}, {}
                for pair in ((0, 1), (2, 3)):
                  for t4 in pair:
                     A = pspool.tile([128, BCHUNK], F32, tag="ps")
                     bank_mms(A, GH, IMN, [("x", act["xi"])], t4, True, last_stop=False)
                     Bk = pspool.tile([128, BCHUNK], F32, tag="ps")
                     bank_mms(Bk, GH, IM, [("x", act["xr"])], t4, True, last_stop=False)
                     C = pspool.tile([128, BCHUNK], F32, tag="ps")
                     bank_mms(C, GH, RE, [("x", act["xs"])], t4, True, last_stop=False)
                     bank_mms(A, GH, IMN, [("h", rh_im)], t4, False)
                     bank_mms(Bk, GH, IM, [("h", rh_re)], t4, False)
                     bank_mms(C, GH, RE, [("h", rh_s)], t4, False)
                     pre_r, pre_i = gauss_combine(A, Bk, C, "cp")
                     bre, bim = BT[(GH, 0, t4)], BT[(GH, 1, t4)]
                     cbr = cpool.tile([128, BCHUNK], F16, tag=f"cbr{t4%2}")
                     cbi = cpool.tile([128, BCHUNK], F16, tag=f"cbi{t4%2}")
                     nc.scalar.activation(cbr[:], pre_r[:], AF.Identity, bias=bre[:])
                     nc.scalar.activation(cbi[:], pre_i[:], AF.Identity, bias=bim[:])
                     sre = spool.tile([128, BCHUNK], F16, tag="sre")
                     sim_ = spool.tile([128, BCHUNK], F16, tag="sim")
                     nc.scalar.activation(sre[:], pre_r[:], AF.Square, bias=bre[:])
                     nc.scalar.activation(sim_[:], pre_i[:], AF.Square, bias=bim[:])
                     m2 = spool.tile([128, BCHUNK], F16, tag="m2")
                     nc.vector.tensor_tensor(m2[:], sre[:], sim_[:], ALU.add)
                     # mag = exp(0.5 ln m2), 1/mag = exp(-0.5 ln m2): stays in
                     # the natural_log_exp table set (no sqrt set needed; the
                     # custom-DVE recip breaks this walrus build and DVE
                     # iterative reciprocal is ~8x slower)
                     lnm = spool.tile([128, BCHUNK], F32, tag="lnm")
                     nc.scalar.activation(lnm[:], m2[:], AF.Ln)
                     mag = cpool.tile([128, BCHUNK], F16, tag=f"mag{t4%2}")
                     nc.scalar.activation(mag[:], lnm[:], AF.Exp, scale=0.5)
                     inv = cpool.tile([128, BCHUNK], F16, tag=f"inv{t4%2}")
                     nc.scalar.activation(inv[:], lnm[:], AF.Exp, scale=-0.5)
                     cb16[t4], mag16[t4], inv16[t4] = (cbr, cbi), mag, inv

                  # beta pass: Tanh (sigmoid/tanh table set -- same set the
                  # next chunk's sigmoids use) + blend h_new = h+z*(h_tilde-h)
                  for t4 in pair:
                    cbr, cbi = cb16[t4]
                    th = spool.tile([128, BCHUNK], F16, tag="th")
                    nc.scalar.activation(th[:], mag16[t4][:], AF.Tanh)
                    tf = spool.tile([128, BCHUNK], F16, tag="tf")
                    nc.vector.tensor_tensor(tf[:], th[:], inv16[t4][:], ALU.mult)
                    htr = spool.tile([128, BCHUNK], F16, tag="htr")
                    hti = spool.tile([128, BCHUNK], F16, tag="hti")
                    nc.vector.tensor_tensor(htr[:], tf[:], cbr[:], ALU.mult)
                    nc.vector.tensor_tensor(hti[:], tf[:], cbi[:], ALU.mult)

                    # final: h_new = h + z*(h_tilde - h)
                    dre = spool.tile([128, BCHUNK], F16, tag="dre")
                    dim = spool.tile([128, BCHUNK], F16, tag="dim")
                    nc.vector.tensor_tensor(dre[:], htr[:], act["hr"][t4][:], ALU.subtract)
                    nc.vector.tensor_tensor(dim[:], hti[:], act["hi"][t4][:], ALU.subtract)
                    zr, zi = z16[(t4, 0)], z16[(t4, 1)]
                    u1 = spool.tile([128, BCHUNK], F16, tag="u1")
                    u2 = spool.tile([128, BCHUNK], F16, tag="u2")
                    nc.vector.tensor_tensor(u1[:], zr[:], dre[:], ALU.mult)
                    nc.vector.tensor_tensor(u2[:], zi[:], dim[:], ALU.mult)
                    ere = spool.tile([128, BCHUNK], F16, tag="ere")
                    nc.vector.tensor_tensor(ere[:], u1[:], u2[:], ALU.subtract)
                    u3 = spool.tile([128, BCHUNK], F16, tag="u1")
                    u4 = spool.tile([128, BCHUNK], F16, tag="u2")
                    nc.vector.tensor_tensor(u3[:], zr[:], dim[:], ALU.mult)
                    nc.vector.tensor_tensor(u4[:], zi[:], dre[:], ALU.mult)
                    eim = spool.tile([128, BCHUNK], F16, tag="eim")
                    nc.vector.tensor_tensor(eim[:], u3[:], u4[:], ALU.add)
                    orr = opool.tile([128, BCHUNK], F32, tag="or")
                    oii = opool.tile([128, BCHUNK], F32, tag="oi")
                    nc.vector.tensor_tensor(orr[:], act["hr"][t4][:], ere[:], ALU.add)
                    nc.vector.tensor_tensor(oii[:], act["hi"][t4][:], eim[:], ALU.add)
                    nc.sync.dma_start(outr[t4 * 128:(t4 + 1) * 128, bsl], orr[:])
                    nc.sync.dma_start(outi[t4 * 128:(t4 + 1) * 128, bsl], oii[:])

    if split_for_hw:
        _split_waits(nc)
    return nc


def _prep(inputs):
    x_re, x_im = inputs["x_re"], inputs["x_im"]
    h_re, h_im = inputs["h_re"], inputs["h_im"]

    def actT(a, sl):
        return np.ascontiguousarray(
            a[sl].T.reshape(4, 128, B_LOCAL).astype(np.float16))

    def wvar(Wre, Wim):
        out = np.empty((3, 3, 4, 128, 512), np.float16)
        for g in range(3):
            WreT, WimT = Wre[g].T, Wim[g].T
            out[g, RE] = WreT.reshape(4, 128, 512)
            out[g, IM] = (WimT - WreT).reshape(4, 128, 512)
            out[g, IMN] = (-(WreT + WimT)).reshape(4, 128, 512)
        return out

    wxn = wvar(inputs["Wx_re"], inputs["Wx_im"])
    whn = wvar(inputs["Wh_re"], inputs["Wh_im"])
    bias = np.stack([inputs["bx_re"] + inputs["bh_re"],
                     inputs["bx_im"] + inputs["bh_im"]],
                    axis=1).reshape(3, 2, 4, 128).astype(np.float32)
    x_s = x_re + x_im
    h_s = h_re + h_im

    in_maps = []
    for c in range(N_CORES):
        sl = slice(c * B_LOCAL, (c + 1) * B_LOCAL)
        in_maps.append({
            "xr": actT(x_re, sl), "xi": actT(x_im, sl), "xs": actT(x_s, sl),
            "hr": actT(h_re, sl), "hi": actT(h_im, sl), "hs": actT(h_s, sl),
            "wx": wxn, "wh": whn, "bias": bias,
        })
    return in_maps


def kernel(**inputs):
    if "nc" not in _CACHE:
        nc = _build(split_for_hw=False)
        try:
            from concourse.timeline_sim import TimelineSim
            LAST_RUN_INFO["timeline_ns"] = int(TimelineSim(nc).simulate())
        except Exception:
            pass
        _CACHE["nc"] = _split_waits(nc)
    nc = _CACHE["nc"]

    in_maps = _prep(inputs)
    res = run_bass_kernel_spmd(nc, in_maps, list(range(N_CORES)))
    LAST_RUN_INFO["exec_time_ns"] = res.exec_time_ns

    out = np.empty((B_FULL, 512, 2), np.float32)
    for c, r in enumerate(res.results):
        sl = slice(c * B_LOCAL, (c + 1) * B_LOCAL)
        out[sl, :, 0] = r["outr"].T
        out[sl, :, 1] = r["outi"].T
    return out



# revision 27
# speedup vs baseline: 1.2688x; 1.2688x over previous
"""Complex GRU cell on 8 Trainium2 NeuronCores (Bass/Tile).

Strategy
  - Data-parallel: batch 16384 -> 8 cores x 2048; 512x512 weights replicated.
  - Feature-major (transposed) layout on device: host pre-transposes x,h to
    [128, 4, B_local] fp16 and pre-transposes the weights, so the kernel needs
    no on-device transposes and per-feature biases land on SBUF partitions
    (fused into ACT ops).
  - Complex matmul uses the Gauss 3-multiplication trick: with host-prepared
    weight variants Wr, (Wi-Wr), -(Wr+Wi) and input sums (Xr+Xi),
      C = (Xr+Xi)@Wr, A = Xi@(-(Wr+Wi)), B = Xr@(Wi-Wr)
      re = C + A, im = C + B
    i.e. 3 fp16 matmul groups (fp32 PSUM accumulate) + one ACT copy + two DVE
    adds, instead of 4 matmul groups -- 25% less TensorE work.
  - DMA batching: weights packed per-gate in first-use order (R, Z, H) and
    loaded with ONE descriptor per gate; acts one DMA per type per chunk;
    biases one [128, 24] DMA; outputs f16, one DMA per t4 block.  The HWDGE
    fixed cost (~625ns per dma_start, serialized) made the old 224-DMA
    schedule spend ~69us before the first matmul.
  - Per batch-chunk of 512: R wave (sigmoid gate), r*h on DVE, C wave
    (x part + (r*h) part) with polar-tanh via ACT Square/Ln/Exp, then the
    Z wave (sigmoid), final complex blend h + z*(h~ - h).  Z runs LAST so
    the final chunk's exposed tail after the last matmul is only the short
    sigmoid+blend chain, not the deep polar-tanh chain.
  - The C-wave alpha passes for all four t4 blocks run back to back (and the
    beta/Tanh passes after), so each chunk costs only 2 ACT table-set
    switches on hardware instead of 4.
  - Two of the blend multiplies run on the otherwise-idle GpSimd (Pool)
    engine; f16 outputs halve the store traffic.  All scheduling variants
    (F_* flags below) were swept with the TimelineSim cost model.
"""
import sys

for _p in ("/opt/trn_rl_repo",):
    if _p not in sys.path:
        sys.path.insert(0, _p)

import numpy as np
import concourse.bass as bass
import concourse.tile as tile
import concourse.mybir as mybir
from concourse.bass_utils import run_bass_kernel_spmd

F32, F16 = mybir.dt.float32, mybir.dt.float16
AF = mybir.ActivationFunctionType
ALU = mybir.AluOpType

RE, IM, IMN = 0, 1, 2  # weight variant slots: Wr, (Wi-Wr), -(Wr+Wi)
RS, ZS, HS = 0, 1, 2   # gate slots in first-use order (R, Z, H)

N_CORES = 8
B_FULL, D, H = 16384, 512, 512
B_LOCAL = B_FULL // N_CORES
BCHUNK = 512
NBC = B_LOCAL // BCHUNK

LAST_RUN_INFO = {}
_CACHE = {}

# scheduling variants (sim-swept)
F_XP = 1          # C-wave x-part lookahead depth in t4 (0/1/2; sim-swept)
F_ZMERGE = False  # blend after the Z wave (sim-swept best with POOLBLEND)
F_DHOIST = True   # dre/dim computed in the beta pass instead of the blend
F_NOCSB = False   # dual-PSUM DVE read REJECTED by walrus verifier; keep ACT copy
F_POOLBLEND = True   # u3/u4 of the blend on the GpSimd (Pool) engine
F_POOLRH = False  # t3/t4b/rhi of the r*h chain on Pool (shortens DVE queue)
F_WARMUP = False  # dummy matmul stream during the initial DMA wait so the
                  # PE p-state ramp (0.65/1.2 GHz for the first 3us of busy)
                  # completes before real matmuls arrive


def _split_waits(nc, maxw=1):
    """walrus here allows 1 sync wait per instruction; hoist extras onto NoOps."""
    for fn in nc.m.functions:
        for bb in fn.blocks:
            out = []
            for inst in list(bb.instructions):
                si = inst.sync_info
                waits = list(si.on_wait) if si is not None else []
                if len(waits) > maxw:
                    extra, keep = waits[:-maxw], waits[-maxw:]
                    k = 0
                    while extra:
                        chunk, extra = extra[:maxw], extra[maxw:]
                        out.append(mybir.InstNoOp(
                            name=f"{inst.name}-wsplit{k}", engine=inst.engine,
                            ins=[], outs=[],
                            sync_info=mybir.SyncInfo(on_wait=chunk, on_update=[])))
                        k += 1
                    inst.sync_info = mybir.SyncInfo(on_wait=keep,
                                                    on_update=list(si.on_update))
                out.append(inst)
            bb.instructions[:] = out
    return nc


def _build(split_for_hw=True):
    nc = bass.Bass("TRN2", target_bir_lowering=False, debug=False)

    # acts: [128, 4, B_LOCAL] fp16, partition-major so chunked loads are one
    # DMA with 1KiB contiguous rows
    dram_acts = {}
    for nm in ("xi", "hi", "xr", "hr", "xs", "hs"):
        dram_acts[nm] = nc.dram_tensor(nm, [128, 4, B_LOCAL], F16,
                                       kind="ExternalInput")
    # weights: per gate slot (R,Z,H), [128, 24 blocks, 512]; block index =
    # (variant*2 + which)*4 + dt  (which: 0=x, 1=h) -- variant-major so the
    # R-gate load can be split into one DMA per variant, interleaved with the
    # first chunk's act loads (first matmul starts ~7us instead of ~14us)
    wall = nc.dram_tensor("wall", [3, 128, 24, 512], F16, kind="ExternalInput")
    # biases: [128, 24] f32, col = (gslot*2 + comp)*4 + t4
    biasd = nc.dram_tensor("bias", [128, 24], F32, kind="ExternalInput")
    # output: [512, 2, B_LOCAL] f16  (re/im interleaved at dim 1)
    out2 = nc.dram_tensor("out2", [512, 2, B_LOCAL], F16, kind="ExternalOutput")

    with tile.TileContext(nc) as tc:
        with (
            tc.tile_pool(name="wpool", bufs=1) as wpool,
            tc.tile_pool(name="apool", bufs=2) as apool,
            tc.tile_pool(name="rhpool", bufs=1) as rhpool,
            tc.tile_pool(name="zpool", bufs=1) as zpool,
            tc.tile_pool(name="spool", bufs=2) as spool,
            tc.tile_pool(name="opool", bufs=2) as opool,
            tc.tile_pool(name="cpool", bufs=1) as cpool,
            tc.tile_pool(name="pspool", bufs=8, space="PSUM") as pspool,
        ):
            # --- input DMAs, in first-use order ---------------------------
            wg = {}
            wg[RS] = wpool.tile([128, 24 * 512], F16, tag="wgR")
            nc.sync.dma_start(wg[RS][:], wall[RS])
            act_tiles = [None] * NBC

            def load_acts(bc):
                bsl = slice(bc * BCHUNK, (bc + 1) * BCHUNK)
                d = {}
                for nm in ("xi", "hi", "xr", "hr", "xs", "hs"):
                    t = apool.tile([128, 4 * BCHUNK], F16, tag=f"a{nm}",
                                   bufs=(2 if nm in ("hr", "hi") else 1))
                    nc.sync.dma_start(t[:], dram_acts[nm][:, :, bsl])
                    d[nm] = t
                act_tiles[bc] = d

            load_acts(0)
            for gs in (ZS, HS):
                wg[gs] = wpool.tile([128, 24 * 512], F16, tag=f"wg{gs}")
                nc.sync.dma_start(wg[gs][:], wall[gs])
            btile = wpool.tile([128, 24], F32, tag="bias")
            nc.sync.dma_start(btile[:], biasd[:, :])
            for bc in range(1, NBC):
                load_acts(bc)

            if F_WARMUP:
                dumw = wpool.tile([128, 128], F16, tag="dumw")
                dumm = wpool.tile([128, 512], F16, tag="dumm")
                nc.vector.memset(dumw[:], 0.0)
                nc.vector.memset(dumm[:], 0.0)
                dps = pspool.tile([128, 512], F32, tag="ps", name="dps")
                for i in range(64):
                    nc.tensor.matmul(dps[:], dumw[:], dumm[:],
                                     start=(i == 0), stop=(i == 63))
                dscr = spool.tile([128, 512], F16, tag="dscr", bufs=1)
                nc.scalar.activation(dscr[:], dps[:], AF.Identity)

            def Wsl(which, gs, v, dt, t4):
                off = ((which * 3 + v) * 4 + dt) * 512 + t4 * 128
                return wg[gs][:, off:off + 128]

            def Bsl(gs, comp, t4):
                idx = (gs * 2 + comp) * 4 + t4
                return btile[:, idx:idx + 1]

            def asl(act, nm, dt):
                return act[nm][:, dt * BCHUNK:(dt + 1) * BCHUNK]

            def bank_mms(ps, gs, v, srcs, t4, first_start, last_stop=True):
                """One Gauss product group accumulated into psum tile ps.
                srcs: list of (which, get_moving) with get_moving(dt) -> AP."""
                n = len(srcs) * 4
                i = 0
                for which, mov in srcs:
                    for dt in range(4):
                        nc.tensor.matmul(
                            ps[:],
                            Wsl(which, gs, v, dt, t4),
                            mov(dt),
                            start=(first_start and i == 0),
                            stop=(last_stop and i == n - 1))
                        i += 1

            def gauss_combine(A, Bk, C, tagp):
                """re = C + A, im = C + B via one ACT copy + two DVE adds
                (or two direct PSUM+PSUM DVE adds with F_NOCSB)."""
                pre_r = spool.tile([128, BCHUNK], F16, tag=f"{tagp}r")
                pre_i = spool.tile([128, BCHUNK], F16, tag=f"{tagp}i")
                if F_NOCSB:
                    nc.vector.tensor_tensor(pre_r[:], A[:], C[:], ALU.add)
                    nc.vector.tensor_tensor(pre_i[:], Bk[:], C[:], ALU.add)
                else:
                    csb = spool.tile([128, BCHUNK], F16, tag="csb")
                    nc.scalar.activation(csb[:], C[:], AF.Identity)
                    nc.vector.tensor_tensor(pre_r[:], A[:], csb[:], ALU.add)
                    nc.vector.tensor_tensor(pre_i[:], Bk[:], csb[:], ALU.add)
                return pre_r, pre_i

            for bc in range(NBC):
                bsl = slice(bc * BCHUNK, (bc + 1) * BCHUNK)
                act = act_tiles[bc]

                def mx(nm):
                    return lambda dt: asl(act, nm, dt)

                # R wave: r = cv_sigmoid(px1 + pr); rh = r*h (+ sum for Gauss)
                # On the first chunk the IMN weights + xi/hi land first, so
                # the A (IMN) groups for ALL t4 run first, giving the IM/RE
                # weight+act DMAs a ~7us runway.
                rh_re, rh_im, rh_s = {}, {}, {}
                for t4 in range(4):
                    A = pspool.tile([128, BCHUNK], F32, tag="ps")
                    bank_mms(A, RS, IMN, [(0, mx("xi")), (1, mx("hi"))], t4, True)
                    Bk = pspool.tile([128, BCHUNK], F32, tag="ps")
                    bank_mms(Bk, RS, IM, [(0, mx("xr")), (1, mx("hr"))], t4, True)
                    C = pspool.tile([128, BCHUNK], F32, tag="ps")
                    bank_mms(C, RS, RE, [(0, mx("xs")), (1, mx("hs"))], t4, True)
                    pre_r, pre_i = gauss_combine(A, Bk, C, "rp")
                    rr = spool.tile([128, BCHUNK], F16, tag="r0")
                    ri = spool.tile([128, BCHUNK], F16, tag="r1")
                    nc.scalar.activation(rr[:], pre_r[:], AF.Sigmoid,
                                         bias=Bsl(RS, 0, t4))
                    nc.scalar.activation(ri[:], pre_i[:], AF.Sigmoid,
                                         bias=Bsl(RS, 1, t4))
                    t1 = spool.tile([128, BCHUNK], F16, tag="t1")
                    t2 = spool.tile([128, BCHUNK], F16, tag="t2")
                    nc.vector.tensor_tensor(t1[:], rr[:], asl(act, "hr", t4), ALU.mult)
                    nc.vector.tensor_tensor(t2[:], ri[:], asl(act, "hi", t4), ALU.mult)
                    rhr = rhpool.tile([128, BCHUNK], F16, tag=f"rhr{t4}")
                    nc.vector.tensor_tensor(rhr[:], t1[:], t2[:], ALU.subtract)
                    t3 = spool.tile([128, BCHUNK], F16, tag="t3", bufs=1)
                    t4b = spool.tile([128, BCHUNK], F16, tag="t4b", bufs=1)
                    eng_rh = nc.gpsimd if F_POOLRH else nc.vector
                    eng_rh.tensor_tensor(t3[:], rr[:], asl(act, "hi", t4), ALU.mult)
                    eng_rh.tensor_tensor(t4b[:], ri[:], asl(act, "hr", t4), ALU.mult)
                    rhi = rhpool.tile([128, BCHUNK], F16, tag=f"rhi{t4}")
                    eng_rh.tensor_tensor(rhi[:], t3[:], t4b[:], ALU.add)
                    rhs = rhpool.tile([128, BCHUNK], F16, tag=f"rhs{t4}")
                    nc.vector.tensor_tensor(rhs[:], rhr[:], rhi[:], ALU.add)
                    rh_re[t4], rh_im[t4], rh_s[t4] = rhr, rhi, rhs

                def mrh(d):
                    return lambda dt: d[dt][:]

                # C wave: c = px2 + (r*h)@Wh2^T; h_tilde = polar_tanh(c + b)
                # alpha pass: everything through Ln/Exp (one table-set switch
                # into natural_log_exp; Identity/Square are fillers in every
                # set so they don't thrash the ACT tables)
                cb16, mag16, inv16 = {}, {}, {}
                ht16 = {}

                def c_xparts(t4):
                    A = pspool.tile([128, BCHUNK], F32, tag="ps")
                    bank_mms(A, HS, IMN, [(0, mx("xi"))], t4, True, last_stop=False)
                    Bk = pspool.tile([128, BCHUNK], F32, tag="ps")
                    bank_mms(Bk, HS, IM, [(0, mx("xr"))], t4, True, last_stop=False)
                    C = pspool.tile([128, BCHUNK], F32, tag="ps")
                    bank_mms(C, HS, RE, [(0, mx("xs"))], t4, True, last_stop=False)
                    return A, Bk, C

                # 2-t4 lookahead on the x-only groups so the h-part matmuls
                # never stall on the R wave's r*h DVE chain
                nxp = int(F_XP)
                xp = {k: c_xparts(k) for k in range(nxp)}
                for t4 in range(4):
                     if t4 in xp:
                         A, Bk, C = xp.pop(t4)
                     else:
                         A, Bk, C = c_xparts(t4)
                     if t4 + nxp < 4 and nxp:
                         xp[t4 + nxp] = c_xparts(t4 + nxp)
                     bank_mms(A, HS, IMN, [(1, mrh(rh_im))], t4, False)
                     bank_mms(Bk, HS, IM, [(1, mrh(rh_re))], t4, False)
                     bank_mms(C, HS, RE, [(1, mrh(rh_s))], t4, False)
                     pre_r, pre_i = gauss_combine(A, Bk, C, "cp")
                     bre, bim = Bsl(HS, 0, t4), Bsl(HS, 1, t4)
                     cbr = cpool.tile([128, BCHUNK], F16, tag=f"cbr{t4}")
                     cbi = cpool.tile([128, BCHUNK], F16, tag=f"cbi{t4}")
                     nc.scalar.activation(cbr[:], pre_r[:], AF.Identity, bias=bre)
                     nc.scalar.activation(cbi[:], pre_i[:], AF.Identity, bias=bim)
                     sre = spool.tile([128, BCHUNK], F16, tag="sre")
                     sim_ = spool.tile([128, BCHUNK], F16, tag="sim")
                     nc.scalar.activation(sre[:], pre_r[:], AF.Square, bias=bre)
                     nc.scalar.activation(sim_[:], pre_i[:], AF.Square, bias=bim)
                     m2 = spool.tile([128, BCHUNK], F16, tag="m2")
                     nc.vector.tensor_tensor(m2[:], sre[:], sim_[:], ALU.add)
                     # mag = exp(0.5 ln m2), 1/mag = exp(-0.5 ln m2): stays in
                     # the natural_log_exp table set (no sqrt set needed; the
                     # custom-DVE recip breaks this walrus build and DVE
                     # iterative reciprocal is ~8x slower)
                     lnm = spool.tile([128, BCHUNK], F16, tag="lnm")
                     nc.scalar.activation(lnm[:], m2[:], AF.Ln)
                     mag = cpool.tile([128, BCHUNK], F16, tag=f"mag{t4}")
                     nc.scalar.activation(mag[:], lnm[:], AF.Exp, scale=0.5)
                     inv = cpool.tile([128, BCHUNK], F16, tag=f"inv{t4}")
                     nc.scalar.activation(inv[:], lnm[:], AF.Exp, scale=-0.5)
                     cb16[t4], mag16[t4], inv16[t4] = (cbr, cbi), mag, inv

                # beta pass: Tanh (sigmoid/tanh table set -- same set the
                # Z-wave sigmoids and next chunk's R sigmoids use). Unpaired
                # from alpha so each chunk does only 2 table-set switches.
                for t4 in range(4):
                    cbr, cbi = cb16[t4]
                    th = spool.tile([128, BCHUNK], F16, tag="th")
                    nc.scalar.activation(th[:], mag16[t4][:], AF.Tanh)
                    tf = spool.tile([128, BCHUNK], F16, tag="tf")
                    nc.vector.tensor_tensor(tf[:], th[:], inv16[t4][:], ALU.mult)
                    htr = opool.tile([128, BCHUNK], F16, tag=f"htr{t4}")
                    hti = opool.tile([128, BCHUNK], F16, tag=f"hti{t4}")
                    nc.vector.tensor_tensor(htr[:], tf[:], cbr[:], ALU.mult)
                    nc.vector.tensor_tensor(hti[:], tf[:], cbi[:], ALU.mult)
                    if F_DHOIST:
                        dre = spool.tile([128, BCHUNK], F16, tag=f"dre{t4}", bufs=1)
                        dim = spool.tile([128, BCHUNK], F16, tag=f"dim{t4}", bufs=1)
                        nc.vector.tensor_tensor(dre[:], htr[:], asl(act, "hr", t4), ALU.subtract)
                        nc.vector.tensor_tensor(dim[:], hti[:], asl(act, "hi", t4), ALU.subtract)
                        ht16[t4] = (dre, dim)
                    else:
                        ht16[t4] = (htr, hti)

                # Z wave: z = cv_sigmoid(px0 + pz) -- the LAST matmuls of
                # the chunk, so the deep C-chain above overlaps PE work and
                # the exposed tail is just sigmoid + blend.  The blend is
                # interleaved per t4 so its DVE ops are not queue-blocked
                # behind the last t4's Z combine in the in-order DVE stream.
                for t4 in range(4):
                    if F_ZMERGE == "prelast" and t4 == 3:
                        # emit blends 0-2 before the last Z combine so the
                        # critical zc3->sigmoid->blend3 chain is not queued
                        # behind them on the in-order DVE stream
                        for k in range(3):
                            blend(k)
                    A = pspool.tile([128, BCHUNK], F32, tag="ps")
                    bank_mms(A, ZS, IMN, [(0, mx("xi")), (1, mx("hi"))], t4, True)
                    Bk = pspool.tile([128, BCHUNK], F32, tag="ps")
                    bank_mms(Bk, ZS, IM, [(0, mx("xr")), (1, mx("hr"))], t4, True)
                    C = pspool.tile([128, BCHUNK], F32, tag="ps")
                    bank_mms(C, ZS, RE, [(0, mx("xs")), (1, mx("hs"))], t4, True)
                    pre_r, pre_i = gauss_combine(A, Bk, C, "zp")
                    zr = zpool.tile([128, BCHUNK], F16, tag=f"z{t4}0")
                    zi = zpool.tile([128, BCHUNK], F16, tag=f"z{t4}1")
                    nc.scalar.activation(zr[:], pre_r[:], AF.Sigmoid,
                                         bias=Bsl(ZS, 0, t4))
                    nc.scalar.activation(zi[:], pre_i[:], AF.Sigmoid,
                                         bias=Bsl(ZS, 1, t4))

                    # blend: h_new = h + z*(h_tilde - h), one f16 DMA per t4
                    htr, hti = ht16[t4]
                    dre = spool.tile([128, BCHUNK], F16, tag="dre")
                    dim = spool.tile([128, BCHUNK], F16, tag="dim")
                    nc.vector.tensor_tensor(dre[:], htr[:], asl(act, "hr", t4), ALU.subtract)
                    nc.vector.tensor_tensor(dim[:], hti[:], asl(act, "hi", t4), ALU.subtract)
                    u1 = spool.tile([128, BCHUNK], F16, tag="u1")
                    u2 = spool.tile([128, BCHUNK], F16, tag="u2")
                    nc.vector.tensor_tensor(u1[:], zr[:], dre[:], ALU.mult)
                    nc.vector.tensor_tensor(u2[:], zi[:], dim[:], ALU.mult)
                    ere = spool.tile([128, BCHUNK], F16, tag="ere")
                    nc.vector.tensor_tensor(ere[:], u1[:], u2[:], ALU.subtract)
                    u3 = spool.tile([128, BCHUNK], F16, tag="u3", bufs=1)
                    u4 = spool.tile([128, BCHUNK], F16, tag="u4", bufs=1)
                    eng3 = nc.gpsimd if F_POOLBLEND else nc.vector
                    eng3.tensor_tensor(u3[:], zr[:], dim[:], ALU.mult)
                    eng3.tensor_tensor(u4[:], zi[:], dre[:], ALU.mult)
                    eim = spool.tile([128, BCHUNK], F16, tag="eim")
                    nc.vector.tensor_tensor(eim[:], u3[:], u4[:], ALU.add)
                    o2 = opool.tile([128, 2 * BCHUNK], F16, tag="o2")
                    nc.vector.tensor_tensor(o2[:, 0:BCHUNK], asl(act, "hr", t4),
                                            ere[:], ALU.add)
                    nc.vector.tensor_tensor(o2[:, BCHUNK:], asl(act, "hi", t4),
                                            eim[:], ALU.add)
                    nc.sync.dma_start(out2[t4 * 128:(t4 + 1) * 128, :, bsl], o2[:])

    if split_for_hw:
        _split_waits(nc)
    return nc


def _prep(inputs):
    x_re, x_im = inputs["x_re"], inputs["x_im"]
    h_re, h_im = inputs["h_re"], inputs["h_im"]

    def actT(a, sl):
        # [B_loc, 512] -> feature-major [128, 4, B_loc] fp16 (f = dt*128 + p)
        t = a[sl].T.reshape(4, 128, B_LOCAL)
        return np.ascontiguousarray(np.moveaxis(t, 0, 1).astype(np.float16))

    def wvar(Wre, Wim, g, v):
        # one variant of one gate: [128, 4, 512] fp16 (dt blocks)
        WreT, WimT = Wre[g].T, Wim[g].T
        W = {RE: WreT, IM: WimT - WreT, IMN: -(WreT + WimT)}[v]
        return np.moveaxis(W.reshape(4, 128, 512), 0, 1)  # [p, dt, hout]

    # gate slots in first-use order: R (orig 1), Z (orig 0), H (orig 2);
    # block index = (v*2 + which)*4 + dt
    gorder = {RS: 1, ZS: 0, HS: 2}
    wall = np.empty((3, 128, 24, 512), np.float16)
    for gs, g in gorder.items():
        for v in (RE, IM, IMN):
            wall[gs, :, v * 8:v * 8 + 4, :] =                 wvar(inputs["Wx_re"], inputs["Wx_im"], g, v)
            wall[gs, :, v * 8 + 4:v * 8 + 8, :] =                 wvar(inputs["Wh_re"], inputs["Wh_im"], g, v)

    b_re = (inputs["bx_re"] + inputs["bh_re"]).astype(np.float32)  # [3, 512]
    b_im = (inputs["bx_im"] + inputs["bh_im"]).astype(np.float32)
    bias = np.empty((128, 24), np.float32)
    for gs, g in gorder.items():
        for comp, b in ((0, b_re), (1, b_im)):
            # col = (gs*2+comp)*4 + t4 ; partition p = feature % 128
            bias[:, (gs * 2 + comp) * 4:(gs * 2 + comp) * 4 + 4] = \
                b[g].reshape(4, 128).T
    x_s = x_re + x_im
    h_s = h_re + h_im

    in_maps = []
    for c in range(N_CORES):
        sl = slice(c * B_LOCAL, (c + 1) * B_LOCAL)
        in_maps.append({
            "xr": actT(x_re, sl), "xi": actT(x_im, sl), "xs": actT(x_s, sl),
            "hr": actT(h_re, sl), "hi": actT(h_im, sl), "hs": actT(h_s, sl),
            "wall": wall, "bias": bias,
        })
    return in_maps


def kernel(**inputs):
    if "nc" not in _CACHE:
        nc = _build(split_for_hw=False)
        try:
            from concourse.timeline_sim import TimelineSim
            LAST_RUN_INFO["timeline_ns"] = int(TimelineSim(nc).simulate())
        except Exception:
            pass
        _CACHE["nc"] = _split_waits(nc)
    nc = _CACHE["nc"]

    in_maps = _prep(inputs)
    res = run_bass_kernel_spmd(nc, in_maps, list(range(N_CORES)))
    LAST_RUN_INFO["exec_time_ns"] = res.exec_time_ns

    out = np.empty((B_FULL, 512, 2), np.float32)
    for c, r in enumerate(res.results):
        sl = slice(c * B_LOCAL, (c + 1) * B_LOCAL)
        out[sl, :, 0] = r["out2"][:, 0, :].T
        out[sl, :, 1] = r["out2"][:, 1, :].T
    return out
